# revision 6
# baseline (speedup 1.0000x reference)
import os, sys, types, json

for _p in reversed(os.environ.get("NIX_PYTHONPATH", "").split(os.pathsep)):
    if _p and _p not in sys.path:
        sys.path.insert(0, _p)
if "/opt/trn_rl_repo" not in sys.path:
    sys.path.insert(0, "/opt/trn_rl_repo")

import numpy as np

N = 100000
E = 1600000
B = 64
F = 128
H = 64
A = 5
ROUTE_LEN = 10
EPS = 1e-5
NCORES = 8
NPC = N // NCORES          # 12500 nodes per core
NT = 99                    # col-tiles per core (static)
CPT = 16                   # chunks per tile (self-loops injected separately)
NCHUNK = NT * CPT          # 1768 chunks of 128 edge slots
DUMP = NPC                 # dump row for unused tile rows

LAST_RESULTS = []          # BassKernelResults per launch (for test harness)
_PROG = None


def _install_ntff_hook():
    try:
        import antenv.axon_hooks  # noqa: F401
        return
    except ImportError:
        pass
    try:
        import antenv
        mod = types.ModuleType("antenv.axon_hooks")
        _h = [None]
        mod.set_axon_ntff_profile_hook = lambda h: _h.__setitem__(0, h)
        mod.get_axon_ntff_profile_hook = lambda: _h[0]
        sys.modules["antenv.axon_hooks"] = mod
        antenv.axon_hooks = mod
        from trn_agent_boot.trn_boot import _ntff_profile_via_ctypes
        hook = _ntff_profile_via_ctypes("/opt/axon/libaxon_pjrt.so")
        if hook is not None:
            mod.set_axon_ntff_profile_hook(hook)
    except Exception:
        pass


def _split_multiwaits(nc, limit=1):
    """This walrus build allows only `limit` sem-wait per instruction; hoist
    extras onto preceding EventSemaphore instructions on the same engine."""
    orig = nc.to_json_bytes

    def patched():
        d = json.loads(orig())
        ctr = 0
        for f in d["functions"]:
            for bb in f["blocks"]:
                new = []
                for inst in bb["instructions"]:
                    si = inst.get("sync_info")
                    ow = (si or {}).get("on_wait") or []
                    if len(ow) > limit:
                        for w in ow[:-limit]:
                            ctr += 1
                            new.append({
                                "debug": inst.get("debug"),
                                "engine": inst["engine"],
                                "ins": [],
                                "outs": [],
                                "name": f"antsplitw_{ctr}",
                                "opcode": "EventSemaphore",
                                "sync_info": {"on_update": [], "on_wait": [w]},
                            })
                        si["on_wait"] = ow[-limit:]
                    new.append(inst)
                bb["instructions"] = new
        return json.dumps(d).encode()

    nc.to_json_bytes = patched


def _build_program():
    """One SPMD program: gather-aggregate one GCN layer for this core's
    12500-col shard. out_tiled[t, p, :] = sum_e S[e, tilecol p] * htab[row_e]."""
    global _PROG
    if _PROG is not None:
        return _PROG
    _install_ntff_hook()
    import concourse.bass as bass
    import concourse.mybir as mybir
    from concourse import tile
    from concourse.bass_utils import run_bass_kernel_spmd

    nc = bass.Bass()
    htab_d = nc.declare_dram_parameter("htab", [N, H], mybir.dt.bfloat16, isOutput=False)
    idx_d = nc.declare_dram_parameter("idx", [128, NCHUNK], mybir.dt.int32, isOutput=False)
    ctab_d = nc.declare_dram_parameter("ctab", [128, 2 * NCHUNK + NT], mybir.dt.float32, isOutput=False)
    hown_d = nc.declare_dram_parameter("hown", [NT, 128, H], mybir.dt.bfloat16, isOutput=False)
    zout_d = nc.declare_dram_parameter("zout", [NT, 128, H], mybir.dt.float32, isOutput=True)

    with tile.TileContext(nc) as tc:
        with (
            tc.tile_pool(name="cst", bufs=1) as cst,
            tc.tile_pool(name="mp", bufs=4) as mp,
            tc.tile_pool(name="sp", bufs=16) as sp,
            tc.tile_pool(name="st", bufs=8) as st,
            tc.tile_pool(name="hp", bufs=4) as hp,
            tc.tile_pool(name="ps", bufs=6, space="PSUM") as ps,
        ):
            idx_t = cst.tile([128, NCHUNK], mybir.dt.int32)
            nc.sync.dma_start(idx_t[:, : 2 * CPT], idx_d[:, : 2 * CPT])
            nc.sync.dma_start(idx_t[:, 2 * CPT :], idx_d[:, 2 * CPT :])
            iota_i = cst.tile([128, 128], mybir.dt.int32)
            nc.gpsimd.iota(iota_i[:], pattern=[[1, 128]], base=0, channel_multiplier=0)
            iota_t = cst.tile([128, 128], mybir.dt.bfloat16)
            nc.vector.tensor_copy(iota_t[:], iota_i[:])
            iotac_i = cst.tile([128, 1], mybir.dt.int32)
            nc.gpsimd.iota(iotac_i[:], pattern=[[1, 1]], base=0, channel_multiplier=1)
            iotac_f = cst.tile([128, 1], mybir.dt.float32)
            nc.vector.tensor_copy(iotac_f[:], iotac_i[:])
            ctab_t = cst.tile([128, 2 * NCHUNK + NT], mybir.dt.float32)
            nc.sync.dma_start(ctab_t[:], ctab_d[:])

            for t in range(NT):
                acc = ps.tile([128, H], mybir.dt.float32, space="PSUM")
                msgbuf = mp.tile([128, CPT * H], mybir.dt.bfloat16)
                for j in range(CPT):
                    k = t * CPT + j
                    nc.gpsimd.indirect_dma_start(
                        out=msgbuf[:, j * H : (j + 1) * H],
                        out_offset=None,
                        in_=htab_d[:],
                        in_offset=bass.IndirectOffsetOnAxis(
                            ap=idx_t[:, k : k + 1], axis=0
                        ),
                    )
                # self-loop injection: D[p, c] = dis2[p] * (c == p); h rows
                # pre-tiled by host, streamed via HWDGE (keeps Pool queue free)
                hown_t = hp.tile([128, H], mybir.dt.bfloat16)
                nc.sync.dma_start(hown_t[:], hown_d[t])
                d_t = sp.tile([128, 128], mybir.dt.bfloat16)
                nc.vector.tensor_scalar(
                    out=d_t[:],
                    in0=iota_t[:],
                    scalar1=iotac_f[:, :1],
                    scalar2=ctab_t[:, 2 * NCHUNK + t : 2 * NCHUNK + t + 1],
                    op0=mybir.AluOpType.is_equal,
                    op1=mybir.AluOpType.mult,
                )
                nc.tensor.matmul(
                    acc[:], lhsT=d_t[:], rhs=hown_t[:], start=True, stop=False,
                )
                for j in range(CPT):
                    k = t * CPT + j
                    s_t = sp.tile([128, 128], mybir.dt.bfloat16)
                    nc.vector.tensor_scalar(
                        out=s_t[:],
                        in0=iota_t[:],
                        scalar1=ctab_t[:, k : k + 1],
                        scalar2=ctab_t[:, NCHUNK + k : NCHUNK + k + 1],
                        op0=mybir.AluOpType.is_equal,
                        op1=mybir.AluOpType.mult,
                    )
                    nc.tensor.matmul(
                        acc[:], lhsT=s_t[:], rhs=msgbuf[:, j * H : (j + 1) * H],
                        start=False, stop=(j == CPT - 1),
                    )
                stage = st.tile([128, H], mybir.dt.float32)
                nc.vector.tensor_copy(stage[:], acc[:])
                nc.sync.dma_start(zout_d[t], stage[:])

    _split_multiwaits(nc)

    def launch(maps, trace=False):
        return run_bass_kernel_spmd(nc, maps, list(range(NCORES)), trace=trace)

    _PROG = launch
    return launch


def _pack_edges(row, col, norm):
    """Pack this core's edges (col already 0-based local, sorted by col) into
    NT tiles x CPT chunks x 128 slots. Returns idx[128,NCHUNK] int32,
    ctab[128,2*NCHUNK] f32, dest[NT,128] int32 (node index per tile row)."""
    order = np.argsort(col, kind="stable")
    row, col, norm = row[order], col[order], norm[order]
    ne = len(col)
    counts = np.bincount(col, minlength=NPC)

    cap = NT * CPT * 128
    idx_flat = np.zeros(cap, np.int64)
    coll_flat = np.zeros(cap, np.float32)
    norm_flat = np.zeros(cap, np.float32)
    dest = np.full((NT, 128), DUMP, np.int64)

    t = 0
    pos = 0        # next free slot in current tile (0..2176)
    c_start = 0    # first col of current tile
    e0 = 0         # edge cursor
    TSLOTS = CPT * 128
    for c in range(NPC):
        d = counts[c]
        if (pos + d > TSLOTS) or (c - c_start >= 128):
            t += 1
            pos = 0
            c_start = c
            if t >= NT:
                raise RuntimeError("tile overflow")
        base = t * TSLOTS + pos
        idx_flat[base : base + d] = row[e0 : e0 + d]
        coll_flat[base : base + d] = c - c_start
        norm_flat[base : base + d] = norm[e0 : e0 + d]
        w = c - c_start
        dest[t, w] = c
        pos += d
        e0 += d
    assert e0 == ne

    # chunk-major [p, k] layout: slot s of chunk k sits at [s, k]
    idx2 = idx_flat.reshape(NCHUNK, 128).T.astype(np.int32).copy()
    coll2 = coll_flat.reshape(NCHUNK, 128).T.copy()
    norm2 = norm_flat.reshape(NCHUNK, 128).T.copy()
    return idx2, coll2, norm2, dest


def _prepare(edge_index, edge_weight):
    """Host preprocessing shared by both layers: per-core packed edge tables."""
    import ml_dtypes  # noqa: F401

    row = np.asarray(edge_index[0]).astype(np.int64)
    col = np.asarray(edge_index[1]).astype(np.int64)
    ew = np.asarray(edge_weight, np.float32)
    deg = np.bincount(col, weights=ew.astype(np.float64), minlength=N).astype(np.float32) + 1.0
    dis = 1.0 / np.sqrt(deg)

    # self loops handled via the per-tile diagonal path (not in gather stream)
    norm = dis[row] * ew * dis[col]

    per_core = []
    cid = col // NPC
    for c in range(NCORES):
        m = cid == c
        idx2, coll2, norm2, dest = _pack_edges(row[m], (col[m] - c * NPC), norm[m])
        dis2 = np.zeros((128, NT), np.float32)
        valid = dest < DUMP  # [NT, 128]
        gnode = np.minimum(c * NPC + dest, N - 1)  # [NT, 128]
        dis2[valid.T] = (dis[gnode] ** 2).T[valid.T]
        ctab = np.ascontiguousarray(
            np.concatenate([coll2, norm2, dis2], axis=1)
        )
        per_core.append((idx2, ctab, dest, gnode))
    return per_core


def _aggregate_on_hw(h_full_f32, per_core, launch, trace=False):
    """One GCN aggregation layer on 8 cores. h_full [N, H] f32 -> z [N, H] f32."""
    import ml_dtypes

    htab = h_full_f32.astype(ml_dtypes.bfloat16)
    maps = [
        {"htab": htab, "idx": pc[0], "ctab": pc[1],
         "hown": np.ascontiguousarray(htab[pc[3]])}
        for pc in per_core
    ]
    res = launch(maps, trace=trace)
    LAST_RESULTS.append(res)
    z = np.zeros((N, H), np.float32)
    for c in range(NCORES):
        zt = np.asarray(res.results[c]["zout"])  # [NT, 128, H]
        dest = per_core[c][2]
        valid = dest < DUMP
        z[c * NPC + dest[valid]] = zt[valid]
    return z


def _bn(x, g, b):
    m = x.mean(0)
    v = x.var(0)
    return (x - m) / np.sqrt(v + EPS) * g + b


def _host_tail(h2, batch_idx, speed, route,
               sw, sb, sg, sbe, cw, cb, rg, rbe, rw, rb,
               ow1, ob1, og, obe, ow2, ob2):
    batch_idx = np.asarray(batch_idx).astype(np.int64)
    gx = np.full((B, H), -np.inf, np.float32)
    starts = np.searchsorted(batch_idx, np.arange(B), side="left")
    ends = np.searchsorted(batch_idx, np.arange(B), side="right")
    for bi in range(B):
        if ends[bi] > starts[bi]:
            gx[bi] = h2[starts[bi] : ends[bi]].max(0)

    v = np.maximum(_bn(np.asarray(speed) @ sw + sb, sg, sbe), 0.0)

    rt = np.asarray(route).transpose(0, 2, 1)
    rtp = np.pad(rt, ((0, 0), (0, 0), (1, 1)))
    rc = np.zeros((B, ROUTE_LEN), np.float32)
    for dt_ in range(3):
        rc += np.einsum("bit,i->bt", rtp[:, :, dt_ : dt_ + ROUTE_LEN], cw[0, :, dt_])
    rc = rc + cb[0]
    m = rc.mean()
    vv = rc.var()
    rc = (rc - m) / np.sqrt(vv + EPS) * rg[0] + rbe[0]
    rc = np.maximum(rc, 0.0)
    r = rc @ rw + rb

    cat = np.concatenate([gx, v.astype(np.float32), r.astype(np.float32)], axis=1)
    o = np.maximum(_bn(cat @ ow1 + ob1, og, obe), 0.0)
    o = o @ ow2 + ob2
    return np.squeeze(np.asarray(o, np.float32))


def _host_fallback(x, edge_index, edge_weight, b1, W1, g1, be1, W2, b2, g2, be2):
    """Pure-host aggregation path (correctness safety net)."""
    import scipy.sparse as sp

    row = np.asarray(edge_index[0]).astype(np.int64)
    col = np.asarray(edge_index[1]).astype(np.int64)
    ew = np.asarray(edge_weight, np.float32)
    deg = np.bincount(col, weights=ew.astype(np.float64), minlength=N).astype(np.float32) + 1.0
    dis = 1.0 / np.sqrt(deg)
    loop = np.arange(N, dtype=np.int64)
    rall = np.concatenate([row, loop])
    call = np.concatenate([col, loop])
    wall = np.concatenate([ew, np.ones(N, np.float32)])
    norm = dis[rall] * wall * dis[call]
    Amat = sp.csr_matrix((norm, (call, rall)), shape=(N, N))

    h = np.maximum(_bn(Amat @ (np.asarray(x) @ W1) + b1, g1, be1), 0.0)
    h2 = np.maximum(_bn(Amat @ (h @ W2) + b2, g2, be2), 0.0)
    return h2


def kernel(x, edge_index, edge_weight, batch_idx, speed, route,
           W1, b1, g1, be1, W2, b2, g2, be2,
           sw, sb, sg, sbe, cw, cb, rg, rbe, rw, rb,
           ow1, ob1, og, obe, ow2, ob2):
    x = np.asarray(x, np.float32)
    trace = bool(os.environ.get("GNN_TRACE"))
    try:
        launch = _build_program()
        per_core = _prepare(edge_index, edge_weight)
        h1 = x @ np.asarray(W1, np.float32)
        z1 = _aggregate_on_hw(h1, per_core, launch, trace=trace)
        zb1 = np.maximum(_bn(z1 + b1, g1, be1), 0.0)
        h2in = zb1 @ np.asarray(W2, np.float32)
        z2 = _aggregate_on_hw(h2in, per_core, launch, trace=trace)
        h2 = np.maximum(_bn(z2 + b2, g2, be2), 0.0)
    except Exception:
        import traceback
        traceback.print_exc()
        h2 = _host_fallback(x, edge_index, edge_weight, b1, W1, g1, be1, W2, b2, g2, be2)
    return _host_tail(h2, batch_idx, speed, route,
                      sw, sb, sg, sbe, cw, cb, rg, rbe, rw, rb,
                      ow1, ob1, og, obe, ow2, ob2)


# revision 9
# speedup vs baseline: 1.3784x; 1.3784x over previous
import os, sys, types, json

for _p in reversed(os.environ.get("NIX_PYTHONPATH", "").split(os.pathsep)):
    if _p and _p not in sys.path:
        sys.path.insert(0, _p)
if "/opt/trn_rl_repo" not in sys.path:
    sys.path.insert(0, "/opt/trn_rl_repo")

import numpy as np

N = 100000
E = 1600000
B = 64
F = 128
H = 64
A = 5
ROUTE_LEN = 10
EPS = 1e-5
NCORES = 8
NPC = N // NCORES          # 12500 nodes per core
NT = 143                   # col-tiles per core (static, re-measured for pairing)
CPT = 16                   # chunks per tile (self-loops injected separately)
GPW = CPT // 2             # paired gathers per tile (2 chunks per 256B descriptor)
NCHUNK = NT * CPT          # chunks of 128 edge slots
DUMP = NPC                 # dump row for unused tile rows

LAST_RESULTS = []          # BassKernelResults per launch (for test harness)
_PROG = None


def _install_ntff_hook():
    try:
        import antenv.axon_hooks  # noqa: F401
        return
    except ImportError:
        pass
    try:
        import antenv
        mod = types.ModuleType("antenv.axon_hooks")
        _h = [None]
        mod.set_axon_ntff_profile_hook = lambda h: _h.__setitem__(0, h)
        mod.get_axon_ntff_profile_hook = lambda: _h[0]
        sys.modules["antenv.axon_hooks"] = mod
        antenv.axon_hooks = mod
        from trn_agent_boot.trn_boot import _ntff_profile_via_ctypes
        hook = _ntff_profile_via_ctypes("/opt/axon/libaxon_pjrt.so")
        if hook is not None:
            mod.set_axon_ntff_profile_hook(hook)
    except Exception:
        pass


def _split_multiwaits(nc, limit=1):
    """This walrus build allows only `limit` sem-wait per instruction; hoist
    extras onto preceding EventSemaphore instructions on the same engine."""
    orig = nc.to_json_bytes

    def patched():
        d = json.loads(orig())
        ctr = 0
        for f in d["functions"]:
            for bb in f["blocks"]:
                new = []
                for inst in bb["instructions"]:
                    si = inst.get("sync_info")
                    ow = (si or {}).get("on_wait") or []
                    if len(ow) > limit:
                        for w in ow[:-limit]:
                            ctr += 1
                            new.append({
                                "debug": inst.get("debug"),
                                "engine": inst["engine"],
                                "ins": [],
                                "outs": [],
                                "name": f"antsplitw_{ctr}",
                                "opcode": "EventSemaphore",
                                "sync_info": {"on_update": [], "on_wait": [w]},
                            })
                        si["on_wait"] = ow[-limit:]
                    new.append(inst)
                bb["instructions"] = new
        return json.dumps(d).encode()

    nc.to_json_bytes = patched


def _build_program():
    """One SPMD program: gather-aggregate one GCN layer for this core's
    12500-col shard. out_tiled[t, p, :] = sum_e S[e, tilecol p] * htab[row_e]."""
    global _PROG
    if _PROG is not None:
        return _PROG
    _install_ntff_hook()
    import concourse.bass as bass
    import concourse.mybir as mybir
    from concourse import tile
    from concourse.bass_utils import run_bass_kernel_spmd

    nc = bass.Bass()
    htab_d = nc.declare_dram_parameter("htab", [N + 1, H], mybir.dt.bfloat16, isOutput=False)
    idx_d = nc.declare_dram_parameter("idx", [128, NT * GPW], mybir.dt.int32, isOutput=False)
    ctab_d = nc.declare_dram_parameter("ctab", [128, 2 * NCHUNK + NT], mybir.dt.float32, isOutput=False)
    hown_d = nc.declare_dram_parameter("hown", [NT, 128, H], mybir.dt.bfloat16, isOutput=False)
    zout_d = nc.declare_dram_parameter("zout", [NT, 128, H], mybir.dt.float32, isOutput=True)

    with tile.TileContext(nc) as tc:
        with (
            tc.tile_pool(name="cst", bufs=1) as cst,
            tc.tile_pool(name="mp", bufs=3) as mp,
            tc.tile_pool(name="sp", bufs=16) as sp,
            tc.tile_pool(name="st", bufs=8) as st,
            tc.tile_pool(name="hp", bufs=4) as hp,
            tc.tile_pool(name="ps", bufs=4, space="PSUM") as ps,
        ):
            iota_i = cst.tile([128, 128], mybir.dt.int32)
            nc.gpsimd.iota(iota_i[:], pattern=[[1, 128]], base=0, channel_multiplier=0)
            iota_t = cst.tile([128, 128], mybir.dt.bfloat16)
            nc.vector.tensor_copy(iota_t[:], iota_i[:])
            iotac_i = cst.tile([128, 1], mybir.dt.int32)
            nc.gpsimd.iota(iotac_i[:], pattern=[[1, 1]], base=0, channel_multiplier=1)
            iotac_f = cst.tile([128, 1], mybir.dt.float32)
            nc.vector.tensor_copy(iotac_f[:], iotac_i[:])
            idx_t = cst.tile([128, NT * GPW], mybir.dt.int32)
            nc.sync.dma_start(idx_t[:], idx_d[:])
            ctab_t = cst.tile([128, 2 * NCHUNK + NT], mybir.dt.float32)
            nc.sync.dma_start(ctab_t[:], ctab_d[:])

            for t in range(NT):
                acc = ps.tile([128, H], mybir.dt.float32, space="PSUM")
                msgbuf = mp.tile([128, CPT * H], mybir.dt.bfloat16)
                for g in range(GPW):
                    # one 256B descriptor per partition fetches table rows
                    # (idx, idx+1) = the slot-pair's two edges' rows
                    nc.gpsimd.indirect_dma_start(
                        out=msgbuf[:, 2 * g * H : (2 * g + 2) * H],
                        out_offset=None,
                        in_=htab_d[:],
                        in_offset=bass.IndirectOffsetOnAxis(
                            ap=idx_t[:, t * GPW + g : t * GPW + g + 1], axis=0
                        ),
                    )
                # self-loop injection: D[p, c] = dis2[p] * (c == p); h rows
                # pre-tiled by host, streamed via HWDGE (keeps Pool queue free)
                hown_t = hp.tile([128, H], mybir.dt.bfloat16)
                nc.sync.dma_start(hown_t[:], hown_d[t])
                d_t = sp.tile([128, 128], mybir.dt.bfloat16)
                nc.vector.tensor_scalar(
                    out=d_t[:],
                    in0=iota_t[:],
                    scalar1=iotac_f[:, :1],
                    scalar2=ctab_t[:, 2 * NCHUNK + t : 2 * NCHUNK + t + 1],
                    op0=mybir.AluOpType.is_equal,
                    op1=mybir.AluOpType.mult,
                )
                nc.tensor.matmul(
                    acc[:], lhsT=d_t[:], rhs=hown_t[:], start=True, stop=False,
                )
                for j in range(CPT):
                    k = t * CPT + j
                    s_t = sp.tile([128, 128], mybir.dt.bfloat16)
                    nc.vector.tensor_scalar(
                        out=s_t[:],
                        in0=iota_t[:],
                        scalar1=ctab_t[:, k : k + 1],
                        scalar2=ctab_t[:, NCHUNK + k : NCHUNK + k + 1],
                        op0=mybir.AluOpType.is_equal,
                        op1=mybir.AluOpType.mult,
                    )
                    nc.tensor.matmul(
                        acc[:], lhsT=s_t[:], rhs=msgbuf[:, j * H : (j + 1) * H],
                        start=False, stop=(j == CPT - 1),
                    )
                stage = st.tile([128, H], mybir.dt.float32)
                nc.vector.tensor_copy(stage[:], acc[:])
                nc.sync.dma_start(zout_d[t], stage[:])

    _split_multiwaits(nc)

    def launch(maps, trace=False):
        return run_bass_kernel_spmd(nc, maps, list(range(NCORES)), trace=trace)

    _PROG = launch
    return launch


def _pack_edges(row, col, norm):
    """Pack this core's edges into NT tiles x GPW slot-pairs x 128 partitions.
    Two edges share one 256B gather descriptor when their rows are adjacent
    under a per-core permutation; arcs are strictly row-increasing (acyclic).
    Pairing is done within each destination col's edge group (plus a carry),
    so slot-pair usage is known incrementally and tiles close exactly."""
    order = np.argsort(col, kind="stable")
    row, col, norm = row[order], col[order], norm[order]
    counts = np.bincount(col, minlength=NPC)
    starts = np.concatenate([[0], np.cumsum(counts)])

    nxt = np.full(N, -1, np.int64)
    prv = np.full(N, -1, np.int64)

    SP = GPW * 128
    rowsA = np.zeros((NT, SP), np.int64)
    cA = np.zeros((NT, SP), np.float32); nA = np.zeros((NT, SP), np.float32)
    cB = np.zeros((NT, SP), np.float32); nB = np.zeros((NT, SP), np.float32)
    dest = np.full((NT, 128), DUMP, np.int64)
    dis2_first = np.zeros((NT, 128), bool)
    seen_col = np.zeros(NPC, bool)

    t = 0
    sp = 0          # next slot-pair in tile t
    c_start = 0
    carry = None    # (row, col_local, norm) unpaired edge awaiting a partner

    def place_pair(a, b):
        nonlocal sp
        rowsA[t, sp] = a[0]
        cA[t, sp] = a[1]; nA[t, sp] = a[2]
        if b is not None:
            cB[t, sp] = b[1]; nB[t, sp] = b[2]
        sp += 1

    for c in range(NPC):
        d = counts[c]
        if d == 0:
            continue
        # close tile if col range exceeded or worst-case slots insufficient
        need_worst = d + (1 if carry else 0)
        if (sp > 0 and (c - c_start >= 128 or sp + need_worst > SP)):
            if carry is not None:
                place_pair(carry, None)
                carry = None
            t += 1
            sp = 0
            c_start = c
            if t >= NT:
                raise RuntimeError("tile overflow")
        cl = c - c_start
        dest[t, cl] = c
        if not seen_col[c]:
            dis2_first[t, cl] = True
            seen_col[c] = True
        e0, e1 = starts[c], starts[c + 1]
        er = np.sort(row[e0:e1])
        o = np.argsort(row[e0:e1], kind="stable")
        grp = [(row[e0 + i], cl, norm[e0 + i]) for i in o]
        if carry is not None:
            grp.insert(0, carry)
            carry = None
        i = 0
        while i < len(grp):
            if i + 1 < len(grp):
                a, b = grp[i], grp[i + 1]
                ra, rb = (a, b) if a[0] < b[0] else (b, a)
                if ra[0] != rb[0] and nxt[ra[0]] == -1 and prv[rb[0]] == -1:
                    nxt[ra[0]] = rb[0]; prv[rb[0]] = ra[0]
                    place_pair(ra, rb)
                    i += 2
                    continue
                place_pair(grp[i], None)
                i += 1
            else:
                carry = grp[i]
                i += 1
    if carry is not None:
        place_pair(carry, None)
        carry = None

    # permutation: chains of increasing rows
    perm_order = np.empty(N, np.int64)
    pos = 0
    for h in np.nonzero(prv == -1)[0]:
        r = h
        while r != -1:
            perm_order[pos] = r; pos += 1
            r = nxt[r]
    assert pos == N
    ppos = np.empty(N, np.int64)
    ppos[perm_order] = np.arange(N)

    idx2 = ppos[rowsA]                        # [NT, SP]
    idxg = idx2.reshape(NT, GPW, 128).transpose(2, 0, 1).reshape(128, NT * GPW)
    coll2 = np.zeros((128, NCHUNK), np.float32)
    norm2 = np.zeros((128, NCHUNK), np.float32)
    cA4 = cA.reshape(NT, GPW, 128); nA4 = nA.reshape(NT, GPW, 128)
    cB4 = cB.reshape(NT, GPW, 128); nB4 = nB.reshape(NT, GPW, 128)
    for t_ in range(NT):
        for g in range(GPW):
            coll2[:, t_ * CPT + 2 * g] = cA4[t_, g]
            norm2[:, t_ * CPT + 2 * g] = nA4[t_, g]
            coll2[:, t_ * CPT + 2 * g + 1] = cB4[t_, g]
            norm2[:, t_ * CPT + 2 * g + 1] = nB4[t_, g]
    return (np.ascontiguousarray(idxg.astype(np.int32)), coll2, norm2,
            dest, dis2_first, perm_order)


def _prepare(edge_index, edge_weight):
    """Host preprocessing shared by both layers: per-core packed edge tables."""
    import ml_dtypes  # noqa: F401

    row = np.asarray(edge_index[0]).astype(np.int64)
    col = np.asarray(edge_index[1]).astype(np.int64)
    ew = np.asarray(edge_weight, np.float32)
    deg = np.bincount(col, weights=ew.astype(np.float64), minlength=N).astype(np.float32) + 1.0
    dis = 1.0 / np.sqrt(deg)

    # self loops handled via the per-tile diagonal path (not in gather stream)
    norm = dis[row] * ew * dis[col]

    per_core = []
    cid = col // NPC
    for c in range(NCORES):
        m = cid == c
        idxg, coll2, norm2, dest, dis2_first, perm = _pack_edges(
            row[m], (col[m] - c * NPC), norm[m])
        dis2 = np.zeros((128, NT), np.float32)
        gnode = np.minimum(c * NPC + dest, N - 1)  # [NT, 128]
        valid = (dest < DUMP) & dis2_first
        dis2[valid.T] = (dis[gnode] ** 2).T[valid.T]
        ctab = np.ascontiguousarray(
            np.concatenate([coll2, norm2, dis2], axis=1)
        )
        per_core.append((idxg, ctab, dest, gnode, perm))
    return per_core


def _aggregate_on_hw(h_full_f32, per_core, launch, trace=False):
    """One GCN aggregation layer on 8 cores. h_full [N, H] f32 -> z [N, H] f32."""
    import ml_dtypes

    htab = h_full_f32.astype(ml_dtypes.bfloat16)
    maps = []
    for pc in per_core:
        perm = pc[4]
        htp = np.zeros((N + 1, H), ml_dtypes.bfloat16)
        htp[:N] = htab[perm]
        maps.append({"htab": htp, "idx": pc[0], "ctab": pc[1],
                     "hown": np.ascontiguousarray(htab[pc[3]])})
    res = launch(maps, trace=trace)
    LAST_RESULTS.append(res)
    z = np.zeros((N, H), np.float32)
    for c in range(NCORES):
        zt = np.asarray(res.results[c]["zout"])  # [NT, 128, H]
        dest = per_core[c][2]
        valid = dest < DUMP
        np.add.at(z, c * NPC + dest[valid], zt[valid])
    return z


def _bn(x, g, b):
    m = x.mean(0)
    v = x.var(0)
    return (x - m) / np.sqrt(v + EPS) * g + b


def _host_tail(h2, batch_idx, speed, route,
               sw, sb, sg, sbe, cw, cb, rg, rbe, rw, rb,
               ow1, ob1, og, obe, ow2, ob2):
    batch_idx = np.asarray(batch_idx).astype(np.int64)
    gx = np.full((B, H), -np.inf, np.float32)
    starts = np.searchsorted(batch_idx, np.arange(B), side="left")
    ends = np.searchsorted(batch_idx, np.arange(B), side="right")
    for bi in range(B):
        if ends[bi] > starts[bi]:
            gx[bi] = h2[starts[bi] : ends[bi]].max(0)

    v = np.maximum(_bn(np.asarray(speed) @ sw + sb, sg, sbe), 0.0)

    rt = np.asarray(route).transpose(0, 2, 1)
    rtp = np.pad(rt, ((0, 0), (0, 0), (1, 1)))
    rc = np.zeros((B, ROUTE_LEN), np.float32)
    for dt_ in range(3):
        rc += np.einsum("bit,i->bt", rtp[:, :, dt_ : dt_ + ROUTE_LEN], cw[0, :, dt_])
    rc = rc + cb[0]
    m = rc.mean()
    vv = rc.var()
    rc = (rc - m) / np.sqrt(vv + EPS) * rg[0] + rbe[0]
    rc = np.maximum(rc, 0.0)
    r = rc @ rw + rb

    cat = np.concatenate([gx, v.astype(np.float32), r.astype(np.float32)], axis=1)
    o = np.maximum(_bn(cat @ ow1 + ob1, og, obe), 0.0)
    o = o @ ow2 + ob2
    return np.squeeze(np.asarray(o, np.float32))


def _host_fallback(x, edge_index, edge_weight, b1, W1, g1, be1, W2, b2, g2, be2):
    """Pure-host aggregation path (correctness safety net)."""
    import scipy.sparse as sp

    row = np.asarray(edge_index[0]).astype(np.int64)
    col = np.asarray(edge_index[1]).astype(np.int64)
    ew = np.asarray(edge_weight, np.float32)
    deg = np.bincount(col, weights=ew.astype(np.float64), minlength=N).astype(np.float32) + 1.0
    dis = 1.0 / np.sqrt(deg)
    loop = np.arange(N, dtype=np.int64)
    rall = np.concatenate([row, loop])
    call = np.concatenate([col, loop])
    wall = np.concatenate([ew, np.ones(N, np.float32)])
    norm = dis[rall] * wall * dis[call]
    Amat = sp.csr_matrix((norm, (call, rall)), shape=(N, N))

    h = np.maximum(_bn(Amat @ (np.asarray(x) @ W1) + b1, g1, be1), 0.0)
    h2 = np.maximum(_bn(Amat @ (h @ W2) + b2, g2, be2), 0.0)
    return h2


def kernel(x, edge_index, edge_weight, batch_idx, speed, route,
           W1, b1, g1, be1, W2, b2, g2, be2,
           sw, sb, sg, sbe, cw, cb, rg, rbe, rw, rb,
           ow1, ob1, og, obe, ow2, ob2):
    x = np.asarray(x, np.float32)
    trace = bool(os.environ.get("GNN_TRACE"))
    try:
        launch = _build_program()
        per_core = _prepare(edge_index, edge_weight)
        h1 = x @ np.asarray(W1, np.float32)
        z1 = _aggregate_on_hw(h1, per_core, launch, trace=trace)
        zb1 = np.maximum(_bn(z1 + b1, g1, be1), 0.0)
        h2in = zb1 @ np.asarray(W2, np.float32)
        z2 = _aggregate_on_hw(h2in, per_core, launch, trace=trace)
        h2 = np.maximum(_bn(z2 + b2, g2, be2), 0.0)
    except Exception:
        import traceback
        traceback.print_exc()
        h2 = _host_fallback(x, edge_index, edge_weight, b1, W1, g1, be1, W2, b2, g2, be2)
    return _host_tail(h2, batch_idx, speed, route,
                      sw, sb, sg, sbe, cw, cb, rg, rbe, rw, rb,
                      ow1, ob1, og, obe, ow2, ob2)


# revision 11
# speedup vs baseline: 1.4084x; 1.0218x over previous
import os, sys, types, json

for _p in reversed(os.environ.get("NIX_PYTHONPATH", "").split(os.pathsep)):
    if _p and _p not in sys.path:
        sys.path.insert(0, _p)
if "/opt/trn_rl_repo" not in sys.path:
    sys.path.insert(0, "/opt/trn_rl_repo")

import numpy as np

N = 100000
E = 1600000
B = 64
F = 128
H = 64
A = 5
ROUTE_LEN = 10
EPS = 1e-5
NCORES = 8
NPC = N // NCORES          # 12500 nodes per core
NT = 141                   # col-tiles per core (static, re-measured for pairing)
CPT = 16                   # chunks per tile (self-loops injected separately)
GPW = CPT // 2             # paired gathers per tile (2 chunks per 256B descriptor)
NCHUNK = NT * CPT          # chunks of 128 edge slots
DUMP = NPC                 # dump row for unused tile rows

LAST_RESULTS = []          # BassKernelResults per launch (for test harness)
_PROG = None


def _install_ntff_hook():
    try:
        import antenv.axon_hooks  # noqa: F401
        return
    except ImportError:
        pass
    try:
        import antenv
        mod = types.ModuleType("antenv.axon_hooks")
        _h = [None]
        mod.set_axon_ntff_profile_hook = lambda h: _h.__setitem__(0, h)
        mod.get_axon_ntff_profile_hook = lambda: _h[0]
        sys.modules["antenv.axon_hooks"] = mod
        antenv.axon_hooks = mod
        from trn_agent_boot.trn_boot import _ntff_profile_via_ctypes
        hook = _ntff_profile_via_ctypes("/opt/axon/libaxon_pjrt.so")
        if hook is not None:
            mod.set_axon_ntff_profile_hook(hook)
    except Exception:
        pass


def _split_multiwaits(nc, limit=1):
    """This walrus build allows only `limit` sem-wait per instruction; hoist
    extras onto preceding EventSemaphore instructions on the same engine."""
    orig = nc.to_json_bytes

    def patched():
        d = json.loads(orig())
        ctr = 0
        for f in d["functions"]:
            for bb in f["blocks"]:
                new = []
                for inst in bb["instructions"]:
                    si = inst.get("sync_info")
                    ow = (si or {}).get("on_wait") or []
                    if len(ow) > limit:
                        for w in ow[:-limit]:
                            ctr += 1
                            new.append({
                                "debug": inst.get("debug"),
                                "engine": inst["engine"],
                                "ins": [],
                                "outs": [],
                                "name": f"antsplitw_{ctr}",
                                "opcode": "EventSemaphore",
                                "sync_info": {"on_update": [], "on_wait": [w]},
                            })
                        si["on_wait"] = ow[-limit:]
                    new.append(inst)
                bb["instructions"] = new
        return json.dumps(d).encode()

    nc.to_json_bytes = patched


def _build_program():
    """One SPMD program: gather-aggregate one GCN layer for this core's
    12500-col shard. out_tiled[t, p, :] = sum_e S[e, tilecol p] * htab[row_e]."""
    global _PROG
    if _PROG is not None:
        return _PROG
    _install_ntff_hook()
    import concourse.bass as bass
    import concourse.mybir as mybir
    from concourse import tile
    from concourse.bass_utils import run_bass_kernel_spmd

    nc = bass.Bass()
    htab_d = nc.declare_dram_parameter("htab", [N + 1, H], mybir.dt.bfloat16, isOutput=False)
    idx_d = nc.declare_dram_parameter("idx", [128, NT * GPW], mybir.dt.int32, isOutput=False)
    ctab_d = nc.declare_dram_parameter("ctab", [128, 2 * NCHUNK + NT], mybir.dt.float32, isOutput=False)
    hown_d = nc.declare_dram_parameter("hown", [NT, 128, H], mybir.dt.bfloat16, isOutput=False)
    zout_d = nc.declare_dram_parameter("zout", [NT, 128, H], mybir.dt.float32, isOutput=True)

    with tile.TileContext(nc) as tc:
        with (
            tc.tile_pool(name="cst", bufs=1) as cst,
            tc.tile_pool(name="mp", bufs=3) as mp,
            tc.tile_pool(name="sp", bufs=16) as sp,
            tc.tile_pool(name="st", bufs=8) as st,
            tc.tile_pool(name="hp", bufs=4) as hp,
            tc.tile_pool(name="ps", bufs=4, space="PSUM") as ps,
        ):
            iota_i = cst.tile([128, 128], mybir.dt.int32)
            nc.gpsimd.iota(iota_i[:], pattern=[[1, 128]], base=0, channel_multiplier=0)
            iota_t = cst.tile([128, 128], mybir.dt.bfloat16)
            nc.vector.tensor_copy(iota_t[:], iota_i[:])
            iotac_i = cst.tile([128, 1], mybir.dt.int32)
            nc.gpsimd.iota(iotac_i[:], pattern=[[1, 1]], base=0, channel_multiplier=1)
            iotac_f = cst.tile([128, 1], mybir.dt.float32)
            nc.vector.tensor_copy(iotac_f[:], iotac_i[:])
            idx_t = cst.tile([128, NT * GPW], mybir.dt.int32)
            nc.sync.dma_start(idx_t[:], idx_d[:])
            ctab_t = cst.tile([128, 2 * NCHUNK + NT], mybir.dt.float32)
            nc.sync.dma_start(ctab_t[:], ctab_d[:])

            for t in range(NT):
                acc = ps.tile([128, H], mybir.dt.float32, space="PSUM")
                msgbuf = mp.tile([128, CPT * H], mybir.dt.bfloat16)
                for g in range(GPW):
                    # one 256B descriptor per partition fetches table rows
                    # (idx, idx+1) = the slot-pair's two edges' rows
                    nc.gpsimd.indirect_dma_start(
                        out=msgbuf[:, 2 * g * H : (2 * g + 2) * H],
                        out_offset=None,
                        in_=htab_d[:],
                        in_offset=bass.IndirectOffsetOnAxis(
                            ap=idx_t[:, t * GPW + g : t * GPW + g + 1], axis=0
                        ),
                    )
                # self-loop injection: D[p, c] = dis2[p] * (c == p); h rows
                # pre-tiled by host, streamed via HWDGE (keeps Pool queue free)
                hown_t = hp.tile([128, H], mybir.dt.bfloat16)
                nc.sync.dma_start(hown_t[:], hown_d[t])
                d_t = sp.tile([128, 128], mybir.dt.bfloat16)
                nc.vector.tensor_scalar(
                    out=d_t[:],
                    in0=iota_t[:],
                    scalar1=iotac_f[:, :1],
                    scalar2=ctab_t[:, 2 * NCHUNK + t : 2 * NCHUNK + t + 1],
                    op0=mybir.AluOpType.is_equal,
                    op1=mybir.AluOpType.mult,
                )
                nc.tensor.matmul(
                    acc[:], lhsT=d_t[:], rhs=hown_t[:], start=True, stop=False,
                )
                for j in range(CPT):
                    k = t * CPT + j
                    s_t = sp.tile([128, 128], mybir.dt.bfloat16)
                    nc.vector.tensor_scalar(
                        out=s_t[:],
                        in0=iota_t[:],
                        scalar1=ctab_t[:, k : k + 1],
                        scalar2=ctab_t[:, NCHUNK + k : NCHUNK + k + 1],
                        op0=mybir.AluOpType.is_equal,
                        op1=mybir.AluOpType.mult,
                    )
                    nc.tensor.matmul(
                        acc[:], lhsT=s_t[:], rhs=msgbuf[:, j * H : (j + 1) * H],
                        start=False, stop=(j == CPT - 1),
                    )
                stage = st.tile([128, H], mybir.dt.float32)
                nc.vector.tensor_copy(stage[:], acc[:])
                nc.sync.dma_start(zout_d[t], stage[:])

    _split_multiwaits(nc)

    def launch(maps, trace=False):
        return run_bass_kernel_spmd(nc, maps, list(range(NCORES)), trace=trace)

    _PROG = launch
    return launch


def _pack_edges(row, col, norm):
    """Pack edges into NT tiles x GPW slot-pairs x 128 partitions with
    tile-wide descriptor pairing: candidates = all edges within a 128-col
    window; pair consecutive row-sorted edges whose permutation slots are
    free (arcs strictly increasing -> acyclic); trim whole cols until the
    paired layout fits SP slot-pairs."""
    order = np.argsort(col, kind="stable")
    row, col, norm = row[order], col[order], norm[order]
    ne = len(col)

    nxt = np.full(N, -1, np.int64)
    prv = np.full(N, -1, np.int64)

    SP = GPW * 128
    rowsA = np.zeros((NT, SP), np.int64)
    cA = np.zeros((NT, SP), np.float32); nA = np.zeros((NT, SP), np.float32)
    cB = np.zeros((NT, SP), np.float32); nB = np.zeros((NT, SP), np.float32)
    dest = np.full((NT, 128), DUMP, np.int64)
    dis2_first = np.zeros((NT, 128), bool)
    seen_col = np.zeros(NPC, bool)

    def dry_pair(er):
        """er row-sorted; returns (pairs_idx, singles_idx) without committing."""
        pairs, singles = [], []
        used_s, used_p = set(), set()
        i = 0
        while i < len(er):
            if i + 1 < len(er):
                ra, rb = er[i], er[i + 1]
                if (ra != rb and nxt[ra] == -1 and prv[rb] == -1
                        and ra not in used_s and rb not in used_p):
                    used_s.add(ra); used_p.add(rb)
                    pairs.append((i, i + 1)); i += 2
                    continue
            singles.append(i); i += 1
        return pairs, singles

    t = 0
    i = 0
    while i < ne:
        c_start = col[i]
        j = i
        while j < ne and col[j] < c_start + 128 and (j - i) < 2 * SP:
            j += 1
        while True:
            sub = slice(i, j)
            o = np.argsort(row[sub], kind="stable")
            er = row[sub][o]
            pairs, singles = dry_pair(er)
            if len(pairs) + len(singles) <= SP:
                break
            # drop the last whole col from the candidate window
            last_c = col[j - 1]
            while j > i and col[j - 1] == last_c:
                j -= 1
        # commit
        ec = col[sub][o]; en = norm[sub][o]
        sp = 0
        for a, b in pairs:
            nxt[er[a]] = er[b]; prv[er[b]] = er[a]
            rowsA[t, sp] = er[a]
            cA[t, sp] = ec[a] - c_start; nA[t, sp] = en[a]
            cB[t, sp] = ec[b] - c_start; nB[t, sp] = en[b]
            sp += 1
        for a in singles:
            rowsA[t, sp] = er[a]
            cA[t, sp] = ec[a] - c_start; nA[t, sp] = en[a]
            sp += 1
        for c in np.unique(col[sub]):
            cl = c - c_start
            dest[t, cl] = c
            if not seen_col[c]:
                dis2_first[t, cl] = True
                seen_col[c] = True
        i = j
        t += 1
        if t > NT and i < ne:
            raise RuntimeError("tile overflow")
    if t > NT:
        raise RuntimeError("tile overflow")

    perm_order = np.empty(N, np.int64)
    pos = 0
    for h in np.nonzero(prv == -1)[0]:
        r = h
        while r != -1:
            perm_order[pos] = r; pos += 1
            r = nxt[r]
    assert pos == N
    ppos = np.empty(N, np.int64)
    ppos[perm_order] = np.arange(N)

    idx2 = ppos[rowsA]
    idxg = idx2.reshape(NT, GPW, 128).transpose(2, 0, 1).reshape(128, NT * GPW)
    coll2 = np.zeros((128, NCHUNK), np.float32)
    norm2 = np.zeros((128, NCHUNK), np.float32)
    cA4 = cA.reshape(NT, GPW, 128); nA4 = nA.reshape(NT, GPW, 128)
    cB4 = cB.reshape(NT, GPW, 128); nB4 = nB.reshape(NT, GPW, 128)
    for t_ in range(NT):
        for g in range(GPW):
            coll2[:, t_ * CPT + 2 * g] = cA4[t_, g]
            norm2[:, t_ * CPT + 2 * g] = nA4[t_, g]
            coll2[:, t_ * CPT + 2 * g + 1] = cB4[t_, g]
            norm2[:, t_ * CPT + 2 * g + 1] = nB4[t_, g]
    return (np.ascontiguousarray(idxg.astype(np.int32)), coll2, norm2,
            dest, dis2_first, perm_order)


def _prepare(edge_index, edge_weight):
    """Host preprocessing shared by both layers: per-core packed edge tables."""
    import ml_dtypes  # noqa: F401

    row = np.asarray(edge_index[0]).astype(np.int64)
    col = np.asarray(edge_index[1]).astype(np.int64)
    ew = np.asarray(edge_weight, np.float32)
    deg = np.bincount(col, weights=ew.astype(np.float64), minlength=N).astype(np.float32) + 1.0
    dis = 1.0 / np.sqrt(deg)

    # self loops handled via the per-tile diagonal path (not in gather stream)
    norm = dis[row] * ew * dis[col]

    per_core = []
    cid = col // NPC
    for c in range(NCORES):
        m = cid == c
        idxg, coll2, norm2, dest, dis2_first, perm = _pack_edges(
            row[m], (col[m] - c * NPC), norm[m])
        dis2 = np.zeros((128, NT), np.float32)
        gnode = np.minimum(c * NPC + dest, N - 1)  # [NT, 128]
        valid = (dest < DUMP) & dis2_first
        dis2[valid.T] = (dis[gnode] ** 2).T[valid.T]
        ctab = np.ascontiguousarray(
            np.concatenate([coll2, norm2, dis2], axis=1)
        )
        per_core.append((idxg, ctab, dest, gnode, perm))
    return per_core


def _aggregate_on_hw(h_full_f32, per_core, launch, trace=False):
    """One GCN aggregation layer on 8 cores. h_full [N, H] f32 -> z [N, H] f32."""
    import ml_dtypes

    htab = h_full_f32.astype(ml_dtypes.bfloat16)
    maps = []
    for pc in per_core:
        perm = pc[4]
        htp = np.zeros((N + 1, H), ml_dtypes.bfloat16)
        htp[:N] = htab[perm]
        maps.append({"htab": htp, "idx": pc[0], "ctab": pc[1],
                     "hown": np.ascontiguousarray(htab[pc[3]])})
    res = launch(maps, trace=trace)
    LAST_RESULTS.append(res)
    z = np.zeros((N, H), np.float32)
    for c in range(NCORES):
        zt = np.asarray(res.results[c]["zout"])  # [NT, 128, H]
        dest = per_core[c][2]
        valid = dest < DUMP
        np.add.at(z, c * NPC + dest[valid], zt[valid])
    return z


def _bn(x, g, b):
    m = x.mean(0)
    v = x.var(0)
    return (x - m) / np.sqrt(v + EPS) * g + b


def _host_tail(h2, batch_idx, speed, route,
               sw, sb, sg, sbe, cw, cb, rg, rbe, rw, rb,
               ow1, ob1, og, obe, ow2, ob2):
    batch_idx = np.asarray(batch_idx).astype(np.int64)
    gx = np.full((B, H), -np.inf, np.float32)
    starts = np.searchsorted(batch_idx, np.arange(B), side="left")
    ends = np.searchsorted(batch_idx, np.arange(B), side="right")
    for bi in range(B):
        if ends[bi] > starts[bi]:
            gx[bi] = h2[starts[bi] : ends[bi]].max(0)

    v = np.maximum(_bn(np.asarray(speed) @ sw + sb, sg, sbe), 0.0)

    rt = np.asarray(route).transpose(0, 2, 1)
    rtp = np.pad(rt, ((0, 0), (0, 0), (1, 1)))
    rc = np.zeros((B, ROUTE_LEN), np.float32)
    for dt_ in range(3):
        rc += np.einsum("bit,i->bt", rtp[:, :, dt_ : dt_ + ROUTE_LEN], cw[0, :, dt_])
    rc = rc + cb[0]
    m = rc.mean()
    vv = rc.var()
    rc = (rc - m) / np.sqrt(vv + EPS) * rg[0] + rbe[0]
    rc = np.maximum(rc, 0.0)
    r = rc @ rw + rb

    cat = np.concatenate([gx, v.astype(np.float32), r.astype(np.float32)], axis=1)
    o = np.maximum(_bn(cat @ ow1 + ob1, og, obe), 0.0)
    o = o @ ow2 + ob2
    return np.squeeze(np.asarray(o, np.float32))


def _host_fallback(x, edge_index, edge_weight, b1, W1, g1, be1, W2, b2, g2, be2):
    """Pure-host aggregation path (correctness safety net)."""
    import scipy.sparse as sp

    row = np.asarray(edge_index[0]).astype(np.int64)
    col = np.asarray(edge_index[1]).astype(np.int64)
    ew = np.asarray(edge_weight, np.float32)
    deg = np.bincount(col, weights=ew.astype(np.float64), minlength=N).astype(np.float32) + 1.0
    dis = 1.0 / np.sqrt(deg)
    loop = np.arange(N, dtype=np.int64)
    rall = np.concatenate([row, loop])
    call = np.concatenate([col, loop])
    wall = np.concatenate([ew, np.ones(N, np.float32)])
    norm = dis[rall] * wall * dis[call]
    Amat = sp.csr_matrix((norm, (call, rall)), shape=(N, N))

    h = np.maximum(_bn(Amat @ (np.asarray(x) @ W1) + b1, g1, be1), 0.0)
    h2 = np.maximum(_bn(Amat @ (h @ W2) + b2, g2, be2), 0.0)
    return h2


def kernel(x, edge_index, edge_weight, batch_idx, speed, route,
           W1, b1, g1, be1, W2, b2, g2, be2,
           sw, sb, sg, sbe, cw, cb, rg, rbe, rw, rb,
           ow1, ob1, og, obe, ow2, ob2):
    x = np.asarray(x, np.float32)
    trace = bool(os.environ.get("GNN_TRACE"))
    try:
        launch = _build_program()
        per_core = _prepare(edge_index, edge_weight)
        h1 = x @ np.asarray(W1, np.float32)
        z1 = _aggregate_on_hw(h1, per_core, launch, trace=trace)
        zb1 = np.maximum(_bn(z1 + b1, g1, be1), 0.0)
        h2in = zb1 @ np.asarray(W2, np.float32)
        z2 = _aggregate_on_hw(h2in, per_core, launch, trace=trace)
        h2 = np.maximum(_bn(z2 + b2, g2, be2), 0.0)
    except Exception:
        import traceback
        traceback.print_exc()
        h2 = _host_fallback(x, edge_index, edge_weight, b1, W1, g1, be1, W2, b2, g2, be2)
    return _host_tail(h2, batch_idx, speed, route,
                      sw, sb, sg, sbe, cw, cb, rg, rbe, rw, rb,
                      ow1, ob1, og, obe, ow2, ob2)


# revision 13
# speedup vs baseline: 1.4760x; 1.0480x over previous
import os, sys, types, json

for _p in reversed(os.environ.get("NIX_PYTHONPATH", "").split(os.pathsep)):
    if _p and _p not in sys.path:
        sys.path.insert(0, _p)
if "/opt/trn_rl_repo" not in sys.path:
    sys.path.insert(0, "/opt/trn_rl_repo")

import numpy as np

N = 100000
E = 1600000
B = 64
F = 128
H = 64
A = 5
ROUTE_LEN = 10
EPS = 1e-5
NCORES = 8
NPC = N // NCORES          # 12500 nodes per core
NT = 133                   # col-tiles per core (static, re-measured for pairing)
CPT = 16                   # chunks per tile (self-loops injected separately)
GPW = CPT // 2             # paired gathers per tile (2 chunks per 256B descriptor)
NCHUNK = NT * CPT          # chunks of 128 edge slots
DUMP = NPC                 # dump row for unused tile rows

LAST_RESULTS = []          # BassKernelResults per launch (for test harness)
_PROG = None


def _install_ntff_hook():
    try:
        import antenv.axon_hooks  # noqa: F401
        return
    except ImportError:
        pass
    try:
        import antenv
        mod = types.ModuleType("antenv.axon_hooks")
        _h = [None]
        mod.set_axon_ntff_profile_hook = lambda h: _h.__setitem__(0, h)
        mod.get_axon_ntff_profile_hook = lambda: _h[0]
        sys.modules["antenv.axon_hooks"] = mod
        antenv.axon_hooks = mod
        from trn_agent_boot.trn_boot import _ntff_profile_via_ctypes
        hook = _ntff_profile_via_ctypes("/opt/axon/libaxon_pjrt.so")
        if hook is not None:
            mod.set_axon_ntff_profile_hook(hook)
    except Exception:
        pass


def _split_multiwaits(nc, limit=1):
    """This walrus build allows only `limit` sem-wait per instruction; hoist
    extras onto preceding EventSemaphore instructions on the same engine."""
    orig = nc.to_json_bytes

    def patched():
        d = json.loads(orig())
        ctr = 0
        for f in d["functions"]:
            for bb in f["blocks"]:
                new = []
                for inst in bb["instructions"]:
                    si = inst.get("sync_info")
                    ow = (si or {}).get("on_wait") or []
                    if len(ow) > limit:
                        for w in ow[:-limit]:
                            ctr += 1
                            new.append({
                                "debug": inst.get("debug"),
                                "engine": inst["engine"],
                                "ins": [],
                                "outs": [],
                                "name": f"antsplitw_{ctr}",
                                "opcode": "EventSemaphore",
                                "sync_info": {"on_update": [], "on_wait": [w]},
                            })
                        si["on_wait"] = ow[-limit:]
                    new.append(inst)
                bb["instructions"] = new
        return json.dumps(d).encode()

    nc.to_json_bytes = patched


def _build_program():
    """One SPMD program: gather-aggregate one GCN layer for this core's
    12500-col shard. out_tiled[t, p, :] = sum_e S[e, tilecol p] * htab[row_e]."""
    global _PROG
    if _PROG is not None:
        return _PROG
    _install_ntff_hook()
    import concourse.bass as bass
    import concourse.mybir as mybir
    from concourse import tile
    from concourse.bass_utils import run_bass_kernel_spmd

    nc = bass.Bass()
    htab_d = nc.declare_dram_parameter("htab", [N + 1, H], mybir.dt.bfloat16, isOutput=False)
    idx_d = nc.declare_dram_parameter("idx", [128, NT * GPW], mybir.dt.int32, isOutput=False)
    ctab_d = nc.declare_dram_parameter("ctab", [128, 2 * NCHUNK + NT], mybir.dt.float32, isOutput=False)
    hown_d = nc.declare_dram_parameter("hown", [NT, 128, H], mybir.dt.bfloat16, isOutput=False)
    zout_d = nc.declare_dram_parameter("zout", [NT, 128, H], mybir.dt.float32, isOutput=True)

    with tile.TileContext(nc) as tc:
        with (
            tc.tile_pool(name="cst", bufs=1) as cst,
            tc.tile_pool(name="mp", bufs=3) as mp,
            tc.tile_pool(name="sp", bufs=16) as sp,
            tc.tile_pool(name="st", bufs=8) as st,
            tc.tile_pool(name="hp", bufs=4) as hp,
            tc.tile_pool(name="ps", bufs=4, space="PSUM") as ps,
        ):
            iota_i = cst.tile([128, 128], mybir.dt.int32)
            nc.gpsimd.iota(iota_i[:], pattern=[[1, 128]], base=0, channel_multiplier=0)
            iota_t = cst.tile([128, 128], mybir.dt.bfloat16)
            nc.vector.tensor_copy(iota_t[:], iota_i[:])
            iotac_i = cst.tile([128, 1], mybir.dt.int32)
            nc.gpsimd.iota(iotac_i[:], pattern=[[1, 1]], base=0, channel_multiplier=1)
            iotac_f = cst.tile([128, 1], mybir.dt.float32)
            nc.vector.tensor_copy(iotac_f[:], iotac_i[:])
            idx_t = cst.tile([128, NT * GPW], mybir.dt.int32)
            nc.sync.dma_start(idx_t[:], idx_d[:])
            ctab_t = cst.tile([128, 2 * NCHUNK + NT], mybir.dt.float32)
            nc.sync.dma_start(ctab_t[:], ctab_d[:])

            for t in range(NT):
                acc = ps.tile([128, H], mybir.dt.float32, space="PSUM")
                msgbuf = mp.tile([128, CPT * H], mybir.dt.bfloat16)
                for g in range(GPW):
                    # one 256B descriptor per partition fetches table rows
                    # (idx, idx+1) = the slot-pair's two edges' rows
                    nc.gpsimd.indirect_dma_start(
                        out=msgbuf[:, 2 * g * H : (2 * g + 2) * H],
                        out_offset=None,
                        in_=htab_d[:],
                        in_offset=bass.IndirectOffsetOnAxis(
                            ap=idx_t[:, t * GPW + g : t * GPW + g + 1], axis=0
                        ),
                    )
                # self-loop injection: D[p, c] = dis2[p] * (c == p); h rows
                # pre-tiled by host, streamed via HWDGE (keeps Pool queue free)
                hown_t = hp.tile([128, H], mybir.dt.bfloat16)
                nc.sync.dma_start(hown_t[:], hown_d[t])
                d_t = sp.tile([128, 128], mybir.dt.bfloat16)
                nc.vector.tensor_scalar(
                    out=d_t[:],
                    in0=iota_t[:],
                    scalar1=iotac_f[:, :1],
                    scalar2=ctab_t[:, 2 * NCHUNK + t : 2 * NCHUNK + t + 1],
                    op0=mybir.AluOpType.is_equal,
                    op1=mybir.AluOpType.mult,
                )
                nc.tensor.matmul(
                    acc[:], lhsT=d_t[:], rhs=hown_t[:], start=True, stop=False,
                )
                for j in range(CPT):
                    k = t * CPT + j
                    s_t = sp.tile([128, 128], mybir.dt.bfloat16)
                    nc.vector.tensor_scalar(
                        out=s_t[:],
                        in0=iota_t[:],
                        scalar1=ctab_t[:, k : k + 1],
                        scalar2=ctab_t[:, NCHUNK + k : NCHUNK + k + 1],
                        op0=mybir.AluOpType.is_equal,
                        op1=mybir.AluOpType.mult,
                    )
                    nc.tensor.matmul(
                        acc[:], lhsT=s_t[:], rhs=msgbuf[:, j * H : (j + 1) * H],
                        start=False, stop=(j == CPT - 1),
                    )
                stage = st.tile([128, H], mybir.dt.float32)
                nc.vector.tensor_copy(stage[:], acc[:])
                nc.sync.dma_start(zout_d[t], stage[:])

    _split_multiwaits(nc)

    def launch(maps, trace=False):
        return run_bass_kernel_spmd(nc, maps, list(range(NCORES)), trace=trace)

    _PROG = launch
    return launch


def _pack_edges(row, col, norm):
    """Pack edges into NT tiles x GPW slot-pairs x 128 partitions with
    tile-wide descriptor pairing: candidates = all edges within a 128-col
    window; pair consecutive row-sorted edges whose permutation slots are
    free (arcs strictly increasing -> acyclic); trim whole cols until the
    paired layout fits SP slot-pairs."""
    order = np.argsort(col, kind="stable")
    row, col, norm = row[order], col[order], norm[order]
    ne = len(col)

    nxt = np.full(N, -1, np.int64)
    prv = np.full(N, -1, np.int64)

    SP = GPW * 128
    rowsA = np.zeros((NT, SP), np.int64)
    cA = np.zeros((NT, SP), np.float32); nA = np.zeros((NT, SP), np.float32)
    cB = np.zeros((NT, SP), np.float32); nB = np.zeros((NT, SP), np.float32)
    dest = np.full((NT, 128), DUMP, np.int64)
    dis2_first = np.zeros((NT, 128), bool)
    seen_col = np.zeros(NPC, bool)

    def dry_pair(er):
        """er row-sorted; returns (pairs_idx, singles_idx) without committing.
        Probes up to 3 positions ahead for a feasible partner."""
        n = len(er)
        taken = bytearray(n)
        pairs, singles = [], []
        used_s, used_p = set(), set()
        for i in range(n):
            if taken[i]:
                continue
            ra = er[i]
            found = False
            if nxt[ra] == -1 and ra not in used_s:
                for j in (i + 1, i + 2, i + 3):
                    if j >= n or taken[j]:
                        continue
                    rb = er[j]
                    if ra != rb and prv[rb] == -1 and rb not in used_p:
                        used_s.add(ra); used_p.add(rb)
                        pairs.append((i, j))
                        taken[i] = taken[j] = 1
                        found = True
                        break
            if not found:
                singles.append(i)
                taken[i] = 1
        return pairs, singles

    t = 0
    i = 0
    while i < ne:
        c_start = col[i]
        j = i
        while j < ne and col[j] < c_start + 128 and (j - i) < 2 * SP:
            j += 1
        while True:
            sub = slice(i, j)
            o = np.argsort(row[sub], kind="stable")
            er = row[sub][o]
            pairs, singles = dry_pair(er)
            if len(pairs) + len(singles) <= SP:
                break
            # drop the last whole col from the candidate window
            last_c = col[j - 1]
            while j > i and col[j - 1] == last_c:
                j -= 1
        # commit
        ec = col[sub][o]; en = norm[sub][o]
        sp = 0
        for a, b in pairs:
            nxt[er[a]] = er[b]; prv[er[b]] = er[a]
            rowsA[t, sp] = er[a]
            cA[t, sp] = ec[a] - c_start; nA[t, sp] = en[a]
            cB[t, sp] = ec[b] - c_start; nB[t, sp] = en[b]
            sp += 1
        for a in singles:
            rowsA[t, sp] = er[a]
            cA[t, sp] = ec[a] - c_start; nA[t, sp] = en[a]
            sp += 1
        for c in np.unique(col[sub]):
            cl = c - c_start
            dest[t, cl] = c
            if not seen_col[c]:
                dis2_first[t, cl] = True
                seen_col[c] = True
        i = j
        t += 1
        if t > NT and i < ne:
            raise RuntimeError("tile overflow")
    if t > NT:
        raise RuntimeError("tile overflow")

    perm_order = np.empty(N, np.int64)
    pos = 0
    for h in np.nonzero(prv == -1)[0]:
        r = h
        while r != -1:
            perm_order[pos] = r; pos += 1
            r = nxt[r]
    assert pos == N
    ppos = np.empty(N, np.int64)
    ppos[perm_order] = np.arange(N)

    idx2 = ppos[rowsA]
    idxg = idx2.reshape(NT, GPW, 128).transpose(2, 0, 1).reshape(128, NT * GPW)
    coll2 = np.zeros((128, NCHUNK), np.float32)
    norm2 = np.zeros((128, NCHUNK), np.float32)
    cA4 = cA.reshape(NT, GPW, 128); nA4 = nA.reshape(NT, GPW, 128)
    cB4 = cB.reshape(NT, GPW, 128); nB4 = nB.reshape(NT, GPW, 128)
    for t_ in range(NT):
        for g in range(GPW):
            coll2[:, t_ * CPT + 2 * g] = cA4[t_, g]
            norm2[:, t_ * CPT + 2 * g] = nA4[t_, g]
            coll2[:, t_ * CPT + 2 * g + 1] = cB4[t_, g]
            norm2[:, t_ * CPT + 2 * g + 1] = nB4[t_, g]
    return (np.ascontiguousarray(idxg.astype(np.int32)), coll2, norm2,
            dest, dis2_first, perm_order)


def _prepare(edge_index, edge_weight):
    """Host preprocessing shared by both layers: per-core packed edge tables."""
    import ml_dtypes  # noqa: F401

    row = np.asarray(edge_index[0]).astype(np.int64)
    col = np.asarray(edge_index[1]).astype(np.int64)
    ew = np.asarray(edge_weight, np.float32)
    deg = np.bincount(col, weights=ew.astype(np.float64), minlength=N).astype(np.float32) + 1.0
    dis = 1.0 / np.sqrt(deg)

    # self loops handled via the per-tile diagonal path (not in gather stream)
    norm = dis[row] * ew * dis[col]

    per_core = []
    cid = col // NPC
    for c in range(NCORES):
        m = cid == c
        idxg, coll2, norm2, dest, dis2_first, perm = _pack_edges(
            row[m], (col[m] - c * NPC), norm[m])
        dis2 = np.zeros((128, NT), np.float32)
        gnode = np.minimum(c * NPC + dest, N - 1)  # [NT, 128]
        valid = (dest < DUMP) & dis2_first
        dis2[valid.T] = (dis[gnode] ** 2).T[valid.T]
        ctab = np.ascontiguousarray(
            np.concatenate([coll2, norm2, dis2], axis=1)
        )
        per_core.append((idxg, ctab, dest, gnode, perm))
    return per_core


def _aggregate_on_hw(h_full_f32, per_core, launch, trace=False):
    """One GCN aggregation layer on 8 cores. h_full [N, H] f32 -> z [N, H] f32."""
    import ml_dtypes

    htab = h_full_f32.astype(ml_dtypes.bfloat16)
    maps = []
    for pc in per_core:
        perm = pc[4]
        htp = np.zeros((N + 1, H), ml_dtypes.bfloat16)
        htp[:N] = htab[perm]
        maps.append({"htab": htp, "idx": pc[0], "ctab": pc[1],
                     "hown": np.ascontiguousarray(htab[pc[3]])})
    res = launch(maps, trace=trace)
    LAST_RESULTS.append(res)
    z = np.zeros((N, H), np.float32)
    for c in range(NCORES):
        zt = np.asarray(res.results[c]["zout"])  # [NT, 128, H]
        dest = per_core[c][2]
        valid = dest < DUMP
        np.add.at(z, c * NPC + dest[valid], zt[valid])
    return z


def _bn(x, g, b):
    m = x.mean(0)
    v = x.var(0)
    return (x - m) / np.sqrt(v + EPS) * g + b


def _host_tail(h2, batch_idx, speed, route,
               sw, sb, sg, sbe, cw, cb, rg, rbe, rw, rb,
               ow1, ob1, og, obe, ow2, ob2):
    batch_idx = np.asarray(batch_idx).astype(np.int64)
    gx = np.full((B, H), -np.inf, np.float32)
    starts = np.searchsorted(batch_idx, np.arange(B), side="left")
    ends = np.searchsorted(batch_idx, np.arange(B), side="right")
    for bi in range(B):
        if ends[bi] > starts[bi]:
            gx[bi] = h2[starts[bi] : ends[bi]].max(0)

    v = np.maximum(_bn(np.asarray(speed) @ sw + sb, sg, sbe), 0.0)

    rt = np.asarray(route).transpose(0, 2, 1)
    rtp = np.pad(rt, ((0, 0), (0, 0), (1, 1)))
    rc = np.zeros((B, ROUTE_LEN), np.float32)
    for dt_ in range(3):
        rc += np.einsum("bit,i->bt", rtp[:, :, dt_ : dt_ + ROUTE_LEN], cw[0, :, dt_])
    rc = rc + cb[0]
    m = rc.mean()
    vv = rc.var()
    rc = (rc - m) / np.sqrt(vv + EPS) * rg[0] + rbe[0]
    rc = np.maximum(rc, 0.0)
    r = rc @ rw + rb

    cat = np.concatenate([gx, v.astype(np.float32), r.astype(np.float32)], axis=1)
    o = np.maximum(_bn(cat @ ow1 + ob1, og, obe), 0.0)
    o = o @ ow2 + ob2
    return np.squeeze(np.asarray(o, np.float32))


def _host_fallback(x, edge_index, edge_weight, b1, W1, g1, be1, W2, b2, g2, be2):
    """Pure-host aggregation path (correctness safety net)."""
    import scipy.sparse as sp

    row = np.asarray(edge_index[0]).astype(np.int64)
    col = np.asarray(edge_index[1]).astype(np.int64)
    ew = np.asarray(edge_weight, np.float32)
    deg = np.bincount(col, weights=ew.astype(np.float64), minlength=N).astype(np.float32) + 1.0
    dis = 1.0 / np.sqrt(deg)
    loop = np.arange(N, dtype=np.int64)
    rall = np.concatenate([row, loop])
    call = np.concatenate([col, loop])
    wall = np.concatenate([ew, np.ones(N, np.float32)])
    norm = dis[rall] * wall * dis[call]
    Amat = sp.csr_matrix((norm, (call, rall)), shape=(N, N))

    h = np.maximum(_bn(Amat @ (np.asarray(x) @ W1) + b1, g1, be1), 0.0)
    h2 = np.maximum(_bn(Amat @ (h @ W2) + b2, g2, be2), 0.0)
    return h2


def kernel(x, edge_index, edge_weight, batch_idx, speed, route,
           W1, b1, g1, be1, W2, b2, g2, be2,
           sw, sb, sg, sbe, cw, cb, rg, rbe, rw, rb,
           ow1, ob1, og, obe, ow2, ob2):
    x = np.asarray(x, np.float32)
    trace = bool(os.environ.get("GNN_TRACE"))
    try:
        launch = _build_program()
        per_core = _prepare(edge_index, edge_weight)
        h1 = x @ np.asarray(W1, np.float32)
        z1 = _aggregate_on_hw(h1, per_core, launch, trace=trace)
        zb1 = np.maximum(_bn(z1 + b1, g1, be1), 0.0)
        h2in = zb1 @ np.asarray(W2, np.float32)
        z2 = _aggregate_on_hw(h2in, per_core, launch, trace=trace)
        h2 = np.maximum(_bn(z2 + b2, g2, be2), 0.0)
    except Exception:
        import traceback
        traceback.print_exc()
        h2 = _host_fallback(x, edge_index, edge_weight, b1, W1, g1, be1, W2, b2, g2, be2)
    return _host_tail(h2, batch_idx, speed, route,
                      sw, sb, sg, sbe, cw, cb, rg, rbe, rw, rb,
                      ow1, ob1, og, obe, ow2, ob2)
